# revision 1
# baseline (speedup 1.0000x reference)
"""BiCutLoss Trainium2 kernel (8-core data parallel over batch).

Reference semantics (B=16384, L=1024):
    temp[b,j]  = argmax(output[b,j,:])          # 1 iff out1 > out0 (ties -> 0)
    idx[b]     = L if row all-ones else index of last zero
    mask[b,j]  = j <= idx[b]
    r1[b,j]    = -1/log2(j+2)  if labels==1 else (j+1)/alpha
    loss       = sum(output[...,1] * mask * r1) / B

Key restructuring: masked_sum = full_sum - tail_sum, where the tail
(j > idx) is confined to the last W columns whenever each row has a zero
decision in its last W positions. For +-symmetric random data
P(no zero in last W=128) = 2^-128 per row; a per-row flag detects the
(cosmically unlikely / adversarial-only) violation and the host falls
back to an exact numpy evaluation, so the kernel is correct for all
inputs. Benefits: out0 is only read on the window (1/8 of it), and the
compare/scan/mask work runs on [128, W] tiles instead of [128, L].

Full sums, two routes balancing VectorE vs TensorE:
  PE route  (most tiles): ql = out1*lab on DVE; colsum(out1), colsum(ql)
             via ones^T-matmul into PSUM; epilogue dots with Bv / D.
  DVE route (a few tiles): r1 = lab*D + Bv materialized against
             partition-broadcast D/Bv tiles; fused (r1*out1) multiply +
             row-sum accumulation in one scalar_tensor_tensor.
Tail sums mirror the same two routes on the window slice.
Host sums the per-core partials and divides by B.
"""

import threading
from contextlib import ExitStack

import numpy as np

B, L = 16384, 1024
N_CORES = 8
ROWS_PER_CORE = B // N_CORES  # 2048
ALPHA = 0.65
W = 64  # tail window width
DVE_ROUTE_TILES = 5  # tiles whose full-sum runs entirely on VectorE

_compiled = threading.local()


def _reward_rows():
    j = np.arange(L, dtype=np.float64)
    bv = (j + 1.0) / ALPHA
    d = -1.0 / np.log2(j + 2.0) - bv
    return bv.astype(np.float32), d.astype(np.float32)


def _build(rows=ROWS_PER_CORE, num_devices=N_CORES, dve_route_tiles=DVE_ROUTE_TILES):
    import concourse.tile as tile
    from concourse import bacc, mybir

    f32 = mybir.dt.float32
    f16 = mybir.dt.float16
    u8 = mybir.dt.uint8
    Alu = mybir.AluOpType
    Act = mybir.ActivationFunctionType

    n_tiles = rows // 128
    n_dve = min(dve_route_tiles, n_tiles)
    n_pe = n_tiles - n_dve

    nc = bacc.Bacc(
        "TRN2",
        target_bir_lowering=False,
        debug=False,
        enable_asserts=True,
        num_devices=num_devices,
    )

    out1_d = nc.dram_tensor("out1", [rows, L], f32, kind="ExternalInput").ap()
    wpack_d = nc.dram_tensor("wpack", [rows, 2 * W], f32, kind="ExternalInput").ap()
    lab_d = nc.dram_tensor("lab", [rows, L], u8, kind="ExternalInput").ap()
    bv_d = nc.dram_tensor("bv", [1, L], f32, kind="ExternalInput").ap()
    dd_d = nc.dram_tensor("dd", [1, L], f32, kind="ExternalInput").ap()
    # partition-broadcast copies for the DVE route ([128, L], same row repeated)
    bvb_d = nc.dram_tensor("bvb", [128, L], f32, kind="ExternalInput").ap()
    ddb_d = nc.dram_tensor("ddb", [128, L], f32, kind="ExternalInput").ap()
    res_d = nc.dram_tensor("partial", [1, 8], f32, kind="ExternalOutput").ap()
    flag_d = nc.dram_tensor("flags", [128, n_tiles], f32, kind="ExternalOutput").ap()
    accs_d = nc.dram_tensor("accs", [128, 2], f32, kind="ExternalOutput").ap()

    with tile.TileContext(nc) as tc, ExitStack() as ctx:
        const = ctx.enter_context(tc.tile_pool(name="const", bufs=1))
        inp = ctx.enter_context(tc.tile_pool(name="inp", bufs=4))
        wpool = ctx.enter_context(tc.tile_pool(name="wpool", bufs=8))
        lpool = ctx.enter_context(tc.tile_pool(name="lpool", bufs=3))
        work = ctx.enter_context(tc.tile_pool(name="work", bufs=5))
        small = ctx.enter_context(tc.tile_pool(name="small", bufs=4))
        psum = ctx.enter_context(tc.tile_pool(name="psum", bufs=1, space="PSUM"))

        ones = const.tile([128, 1], f32)
        nc.vector.memset(ones[:], 1.0)
        bv_row = const.tile([1, L], f32)
        nc.scalar.dma_start(bv_row[:], bv_d[:])
        d_row = const.tile([1, L], f32)
        nc.scalar.dma_start(d_row[:], dd_d[:])
        bvb = const.tile([128, L], f32)
        nc.scalar.dma_start(bvb[:], bvb_d[:])
        ddb = const.tile([128, L], f32)
        nc.scalar.dma_start(ddb[:], ddb_d[:])

        flag_t = const.tile([128, n_tiles], f32)

        # PSUM accumulators: full colsums (PE route) + window tail colsums
        psq_a = psum.tile([1, 512], f32)
        psq_b = psum.tile([1, 512], f32)
        psl_a = psum.tile([1, 512], f32)
        psl_b = psum.tile([1, 512], f32)
        psw_q = psum.tile([1, W], f32)
        psw_l = psum.tile([1, W], f32)

        # DVE-route accumulators
        acc_main = const.tile([128, 1], f32)
        nc.vector.memset(acc_main[:], 0.0)
        acc_tail = const.tile([128, 1], f32)
        nc.vector.memset(acc_tail[:], 0.0)

        import os as _os
        _mode = _os.environ.get("DVE_PLACE", "spread")
        if _mode == "front":
            dve_set = set(range(n_dve))
        else:
            stride = max(1, n_tiles // max(n_dve, 1))
            dve_set = set((k * stride + stride - 1) % n_tiles for k in range(n_dve))
        n_pe_seen = 0
        assert n_tiles % 2 == 0
        pair_tiles = {}
        for i in range(n_tiles):
            if i % 2 == 0:
                r0 = i * 128
                out1_t2 = inp.tile([128, 2 * L], f32, tag="out1p")
                if i == 0:
                    nc.sync.dma_start(out1_t2[:, 0:L], out1_d[r0 : r0 + 128, :])
                    nc.sync.dma_start(out1_t2[:, L : 2 * L], out1_d[r0 + 128 : r0 + 256, :])
                else:
                    nc.sync.dma_start(
                        out1_t2[:].rearrange("p (two l) -> p two l", two=2),
                        out1_d[r0 : r0 + 256, :].rearrange("(two p) l -> p two l", p=128),
                    )
                wpack_t2 = wpool.tile([128, 4 * W], f32, tag="wpackp")
                nc.sync.dma_start(
                    wpack_t2[:].rearrange("p (two l) -> p two l", two=2),
                    wpack_d[r0 : r0 + 256, :].rearrange("(two p) l -> p two l", p=128),
                )
                lab_t2 = lpool.tile([128, 2 * L], u8, tag="labp")
                nc.scalar.dma_start(
                    lab_t2[:].rearrange("p (two l) -> p two l", two=2),
                    lab_d[r0 : r0 + 256, :].rearrange("(two p) l -> p two l", p=128),
                )
                pair_tiles = {"out1": out1_t2, "wpack": wpack_t2, "lab": lab_t2}
            half = i % 2
            dve_route = i in dve_set
            out1_t = pair_tiles["out1"][:, half * L : (half + 1) * L]
            lab_t = pair_tiles["lab"][:, half * L : (half + 1) * L]
            wp = pair_tiles["wpack"][:, half * 2 * W : (half + 1) * 2 * W]
            out0w_t = wp[:, 0:W]
            out1_w = wp[:, W : 2 * W]

            # ---- window mask: ge -> suffix-max s -> tail mask tm ----
            ge_w = work.tile([128, W], f16, tag="gew")
            nc.vector.tensor_tensor(ge_w[:], out0w_t, out1_w, Alu.is_ge)
            s_w = work.tile([128, W], f16, tag="sw")
            nc.vector.tensor_tensor_scan(
                s_w[:, ::-1], ge_w[:, ::-1], ge_w[:, ::-1], 0.0, Alu.max, Alu.max
            )
            # ao = 1 iff no zero decision inside the window (suspicious OR
            # genuinely all-ones row; either way tail contribution -> 0 and
            # the flag lets the host decide).
            nc.vector.tensor_scalar(
                flag_t[:, i : i + 1], s_w[:, 0:1], 0.0, None, Alu.is_equal
            )
            omao_col = small.tile([128, 1], f32, tag="omao")
            nc.vector.tensor_scalar(
                omao_col[:], flag_t[:, i : i + 1], -1.0, 1.0, Alu.mult, Alu.add
            )
            # tm = 1 - s - ao  (1 on the strict tail j > idx, else 0) on ScalarE
            tm_w = work.tile([128, W], f32, tag="tmw")
            nc.scalar.activation(
                tm_w[:], s_w[:], Act.Identity, bias=omao_col[:], scale=-1.0
            )

            if dve_route:
                # r1 = lab*D + Bv ; main = sum_j r1*out1 ; w kept for tail
                t1 = work.tile([128, L], f32, tag="t1")
                nc.vector.tensor_tensor(t1[:], lab_t, ddb[:], Alu.mult)
                r1 = work.tile([128, L], f32, tag="r1")
                nc.vector.tensor_tensor(r1[:], t1[:], bvb[:], Alu.add)
                wfull = work.tile([128, L], f32, tag="wfull")
                row_col = small.tile([128, 1], f32, tag="rowc")
                nc.vector.scalar_tensor_tensor(
                    wfull[:], r1[:], 1.0, out1_t, Alu.mult, Alu.mult,
                    accum_out=row_col[:],
                )
                nc.vector.tensor_tensor(acc_main[:], acc_main[:], row_col[:], Alu.add)
                # tail = sum_jw tm * w_window
                tail_col = small.tile([128, 1], f32, tag="tailc")
                junkw = work.tile([128, W], f32, tag="junkw")
                nc.vector.scalar_tensor_tensor(
                    junkw[:], tm_w[:], 1.0, wfull[:, L - W : L], Alu.mult, Alu.mult,
                    accum_out=tail_col[:],
                )
                nc.vector.tensor_tensor(acc_tail[:], acc_tail[:], tail_col[:], Alu.add)
            else:
                st, sp = n_pe_seen == 0, n_pe_seen == n_pe - 1
                n_pe_seen += 1
                # ql = out1 * lab
                ql = work.tile([128, L], f32, tag="ql")
                nc.vector.tensor_tensor(ql[:], out1_t, lab_t, Alu.mult)
                nc.tensor.matmul(psq_a[:], ones[:], out1_t[:, 0:512], start=st, stop=sp)
                nc.tensor.matmul(psq_b[:], ones[:], out1_t[:, 512:L], start=st, stop=sp)
                nc.tensor.matmul(psl_a[:], ones[:], ql[:, 0:512], start=st, stop=sp)
                nc.tensor.matmul(psl_b[:], ones[:], ql[:, 512:L], start=st, stop=sp)
                # tails: tail_q = tm*out1_w ; tail_ql = tail_q*lab_w
                tq = work.tile([128, W], f32, tag="tq")
                nc.vector.tensor_tensor(tq[:], tm_w[:], out1_w, Alu.mult)
                tl = work.tile([128, W], f32, tag="tl")
                nc.vector.tensor_tensor(tl[:], tq[:], lab_t[:, L - W : L], Alu.mult)
                nc.tensor.matmul(psw_q[:], ones[:], tq[:], start=st, stop=sp)
                nc.tensor.matmul(psw_l[:], ones[:], tl[:], start=st, stop=sp)

        # ---- epilogue: weighted dots straight out of PSUM ----
        res_t = const.tile([1, 8], f32)

        def dot(ps_ap, row_ap, k, tag):
            junk = const.tile([1, ps_ap.shape[1]], f32, tag="junk" + tag)
            nc.vector.scalar_tensor_tensor(
                junk[:], ps_ap, 1.0, row_ap, Alu.mult, Alu.mult,
                accum_out=res_t[0:1, k : k + 1],
            )

        dot(psq_a[:], bv_row[:, 0:512], 0, "1a")
        dot(psq_b[:], bv_row[:, 512:L], 1, "1b")
        dot(psl_a[:], d_row[:, 0:512], 2, "2a")
        dot(psl_b[:], d_row[:, 512:L], 3, "2b")
        dot(psw_q[:], bv_row[:, L - W : L], 4, "3")
        dot(psw_l[:], d_row[:, L - W : L], 5, "4")
        nc.vector.memset(res_t[0:1, 6:8], 0.0)
        nc.scalar.dma_start(res_d[:], res_t[:])
        nc.scalar.dma_start(accs_d[:, 0:1], acc_main[:])
        nc.scalar.dma_start(accs_d[:, 1:2], acc_tail[:])
        nc.scalar.dma_start(flag_d[:], flag_t[:])

    nc.compile()
    return nc


def _get_nc():
    if getattr(_compiled, "nc", None) is None:
        _compiled.nc = _build()
    return _compiled.nc


def _in_maps(output, labels):
    out1 = np.ascontiguousarray(output[:, :, 1], dtype=np.float32)
    wpack = np.empty((B, 2 * W), dtype=np.float32)
    wpack[:, 0:W] = output[:, L - W : L, 0]
    wpack[:, W : 2 * W] = output[:, L - W : L, 1]
    lab = labels.astype(np.uint8)  # values are 0/1
    bv, dd = _reward_rows()
    bvb = np.broadcast_to(bv, (128, L)).copy()
    ddb = np.broadcast_to(dd, (128, L)).copy()
    rp = ROWS_PER_CORE
    return [
        {
            "out1": out1[c * rp : (c + 1) * rp],
            "wpack": wpack[c * rp : (c + 1) * rp],
            "lab": lab[c * rp : (c + 1) * rp],
            "bv": bv.reshape(1, L),
            "dd": dd.reshape(1, L),
            "bvb": bvb,
            "ddb": ddb,
        }
        for c in range(N_CORES)
    ]


def _host_fallback(output, labels):
    temp = output[:, :, 1] > output[:, :, 0]
    allones = temp.all(axis=1)
    z = ~temp
    last_zero = (L - 1) - np.argmax(z[:, ::-1], axis=1)
    idx = np.where(allones, L, last_zero)
    mask = np.arange(L)[None, :] <= idx[:, None]
    j = np.arange(L, dtype=np.float64)
    r1 = np.where(labels == 1, -1.0 / np.log2(j + 2.0), (j + 1.0) / ALPHA)
    return np.float32(
        (output[:, :, 1].astype(np.float64) * mask * r1).sum() / B
    )


def _combine(results, output, labels):
    total = 0.0
    suspicious = 0.0
    for c, r in enumerate(results):
        p = np.asarray(r["partial"], dtype=np.float64)[0]
        total += p[0] + p[1] + p[2] + p[3] - p[4] - p[5]
        accs = np.asarray(r["accs"], dtype=np.float64)
        total += accs[:, 0].sum() - accs[:, 1].sum()
        # rows flagged "no zero in window": genuine all-ones rows are handled
        # (tail = 0) but a row whose last zero is before the window is not —
        # recheck on host. Never fires for +-symmetric random inputs.
        flags = np.asarray(r["flags"], dtype=np.float64)
        if flags.max() > 0:
            rp = ROWS_PER_CORE
            o = output[c * rp : (c + 1) * rp]
            allones_rows = (o[:, :, 1] > o[:, :, 0]).all(axis=1)
            flagged = flags.T.reshape(-1) > 0  # row-major within this core
            suspicious += (flagged & ~allones_rows).sum()
    if suspicious > 0:
        return _host_fallback(output, labels)
    return np.float32(total / B)


def kernel(output: np.ndarray, labels: np.ndarray) -> np.ndarray:
    from concourse.bass_utils import run_bass_kernel_spmd

    assert output.shape == (B, L, 2), output.shape
    nc = _get_nc()
    res = run_bass_kernel_spmd(
        nc, _in_maps(output, labels), core_ids=list(range(N_CORES))
    )
    return _combine(res.results, output, labels)



# revision 9
# speedup vs baseline: 1.0254x; 1.0254x over previous
"""BiCutLoss Trainium2 kernel (8-core data parallel over batch).

Reference semantics (B=16384, L=1024):
    temp[b,j]  = argmax(output[b,j,:])          # 1 iff out1 > out0 (ties -> 0)
    idx[b]     = L if row all-ones else index of last zero
    mask[b,j]  = j <= idx[b]
    r1[b,j]    = -1/log2(j+2)  if labels==1 else (j+1)/alpha
    loss       = sum(output[...,1] * mask * r1) / B

Restructuring: masked_sum = full_sum - tail_sum, with the tail (j > idx)
confined to the last W=64 columns whenever each row has a zero decision
there (P(violation) = 2^-64 per row for symmetric random data; a per-row
flag catches it and the host falls back to exact numpy, so the kernel is
correct for all inputs).

v2 layout (vs f32 baseline):
  * out1 streams as f16 [2048, 1024] (half the bytes, 2x DVE / 4x PE
    rate); labels as u8. Loss tolerance is 2e-2; f16 value quantization
    contributes ~1e-3 via random-sign cancellation.
  * window data stays f32 so the argmax decisions are bit-exact vs the
    reference; it is packed host-side as [128, 16*64] so the whole
    tail-mask computation runs as ~10 full-width ops (iota, segmented
    reduce_max, broadcast compare) instead of 16 fragment chains.
  * full sums: per-tile either DVE route (stt with accum_out into a
    per-tile accumulator column: sum_j out1*Bv and sum_j (lab*D)*out1)
    or PE route (ql = out1*lab on DVE; colsum(out1), colsum(ql) via
    ones^T matmul accumulated in PSUM; f32 epilogue dots with Bv / D).
  * host sums the [128, 34] accumulator columns + [1,8] dots / B.
"""

import threading
from contextlib import ExitStack

import numpy as np

B, L = 16384, 1024
N_CORES = 8
ROWS_PER_CORE = B // N_CORES  # 2048
ALPHA = 0.65
W = 64  # tail window width
N_TILES = ROWS_PER_CORE // 128  # 16
DVE_ROUTE_TILES = 6  # tiles whose full-sum runs entirely on VectorE

_compiled = threading.local()


def _reward_rows():
    j = np.arange(L, dtype=np.float64)
    bv = (j + 1.0) / ALPHA
    d = -1.0 / np.log2(j + 2.0) - bv
    return bv, d


def _build(rows=ROWS_PER_CORE, num_devices=N_CORES, dve_route_tiles=DVE_ROUTE_TILES):
    import concourse.tile as tile
    from concourse import bacc, mybir

    f32 = mybir.dt.float32
    f16 = mybir.dt.float16
    u8 = mybir.dt.uint8
    Alu = mybir.AluOpType
    Ax = mybir.AxisListType

    n_tiles = rows // 128
    n_dve = min(dve_route_tiles, n_tiles)
    n_pe = n_tiles - n_dve

    nc = bacc.Bacc(
        "TRN2",
        target_bir_lowering=False,
        debug=False,
        enable_asserts=True,
        num_devices=num_devices,
    )

    out1_d = nc.dram_tensor("out1", [rows, L], f16, kind="ExternalInput").ap()
    lab_d = nc.dram_tensor("lab", [rows, L], u8, kind="ExternalInput").ap()
    wf_d = nc.dram_tensor("wf", [128, 2 * n_tiles * W], f32, kind="ExternalInput").ap()
    labw_d = nc.dram_tensor("labw", [128, n_tiles * W], u8, kind="ExternalInput").ap()
    rows16_d = nc.dram_tensor("rows16", [2, L], f16, kind="ExternalInput").ap()
    bv32_d = nc.dram_tensor("bv32", [1, L], f32, kind="ExternalInput").ap()
    dd32_d = nc.dram_tensor("dd32", [1, L], f32, kind="ExternalInput").ap()
    res_d = nc.dram_tensor("partial", [1, 8], f32, kind="ExternalOutput").ap()
    accs_d = nc.dram_tensor("accs", [128, 34], f32, kind="ExternalOutput").ap()
    flag_d = nc.dram_tensor("flags", [128, n_tiles], f16, kind="ExternalOutput").ap()

    WSEG = n_tiles * W  # 1024: packed window width

    with tile.TileContext(nc) as tc, ExitStack() as ctx:
        const = ctx.enter_context(tc.tile_pool(name="const", bufs=1))
        inp = ctx.enter_context(tc.tile_pool(name="inp", bufs=4))
        lpool = ctx.enter_context(tc.tile_pool(name="lpool", bufs=4))
        work = ctx.enter_context(tc.tile_pool(name="work", bufs=4))
        junkp = ctx.enter_context(tc.tile_pool(name="junkp", bufs=4))
        psum = ctx.enter_context(tc.tile_pool(name="psum", bufs=1, space="PSUM"))

        # ---- constants ----
        ones16 = const.tile([128, 1], f16)
        nc.vector.memset(ones16[:], 1.0)
        bv_b = const.tile([128, L], f16)
        nc.gpsimd.dma_start(bv_b[:], rows16_d[0:1, :].to_broadcast((128, L)))
        d_b = const.tile([128, L], f16)
        nc.gpsimd.dma_start(d_b[:], rows16_d[1:2, :].to_broadcast((128, L)))
        bv32_t = const.tile([1, L], f32)
        nc.gpsimd.dma_start(bv32_t[:], bv32_d[:])
        dd32_t = const.tile([1, L], f32)
        nc.gpsimd.dma_start(dd32_t[:], dd32_d[:])
        gio = const.tile([128, WSEG], f16)
        nc.gpsimd.iota(
            gio[:],
            [[0, n_tiles], [1, W]],
            base=1,
            channel_multiplier=0,
            allow_small_or_imprecise_dtypes=True,
        )
        accs_t = const.tile([128, 34], f32)
        nc.gpsimd.memset(accs_t[:], 0.0)
        flag_t = const.tile([128, n_tiles], f16)

        # ---- window inputs ----
        wf_t = const.tile([128, 2 * WSEG], f32)
        nc.sync.dma_start(wf_t[:], wf_d[:])
        labw_t = const.tile([128, WSEG], u8)
        nc.scalar.dma_start(labw_t[:], labw_d[:])

        # ---- main input streams: 4 quads of 4 tiles each ----
        out1_q = []
        lab_q = []
        for f in range(n_tiles // 4):
            oq = inp.tile([128, 4, L], f16, tag="out1q")
            nc.sync.dma_start(
                oq[:],
                out1_d[512 * f : 512 * (f + 1), :].rearrange(
                    "(k p) l -> p k l", p=128
                ),
            )
            out1_q.append(oq)
            lq = lpool.tile([128, 4, L], u8, tag="labq")
            nc.scalar.dma_start(
                lq[:],
                lab_d[512 * f : 512 * (f + 1), :].rearrange("(k p) l -> p k l", p=128),
            )
            lab_q.append(lq)

        # ---- window block: tail mask + tail sums, all rows at once ----
        out0w = wf_t[:, 0:WSEG]
        out1w = wf_t[:, WSEG : 2 * WSEG]
        ge = work.tile([128, WSEG], f16, tag="ge")
        nc.vector.tensor_tensor(ge[:], out0w, out1w, Alu.is_ge)
        gi = work.tile([128, WSEG], f16, tag="gi")
        nc.vector.tensor_tensor(gi[:], ge[:], gio[:], Alu.mult)
        last0 = const.tile([128, n_tiles], f16)
        nc.vector.tensor_reduce(
            last0[:],
            gi[:].rearrange("p (s w) -> p s w", w=W),
            Ax.X,
            Alu.max,
        )
        nc.vector.tensor_scalar(flag_t[:], last0[:], 0.5, None, Alu.is_le)
        l0p = const.tile([128, n_tiles], f16)
        nc.vector.scalar_tensor_tensor(
            l0p[:], flag_t[:], float(W), last0[:], Alu.mult, Alu.add
        )
        tm = work.tile([128, WSEG], f16, tag="tm")
        nc.vector.tensor_tensor(
            tm[:].rearrange("p (s w) -> p s w", w=W),
            gio[:].rearrange("p (s w) -> p s w", w=W),
            l0p[:].to_broadcast((128, n_tiles, W)),
            Alu.is_gt,
        )
        u = work.tile([128, WSEG], f16, tag="u")
        nc.vector.tensor_tensor(u[:], tm[:], out1w, Alu.mult)
        bvw_bc = (
            bv_b[:, L - W : L]
            .rearrange("p (s w) -> p s w", s=1)
            .to_broadcast((128, n_tiles, W))
        )
        dw_bc = (
            d_b[:, L - W : L]
            .rearrange("p (s w) -> p s w", s=1)
            .to_broadcast((128, n_tiles, W))
        )
        junk1 = junkp.tile([128, WSEG], f16, tag="junkw")
        nc.vector.scalar_tensor_tensor(
            junk1[:].rearrange("p (s w) -> p s w", w=W),
            u[:].rearrange("p (s w) -> p s w", w=W),
            1.0,
            bvw_bc,
            Alu.mult,
            Alu.mult,
            accum_out=accs_t[:, 32:33],
        )
        q2 = work.tile([128, WSEG], f16, tag="q2")
        nc.vector.tensor_tensor(q2[:], u[:], labw_t[:], Alu.mult)
        junk2 = junkp.tile([128, WSEG], f16, tag="junkw")
        nc.vector.scalar_tensor_tensor(
            junk2[:].rearrange("p (s w) -> p s w", w=W),
            q2[:].rearrange("p (s w) -> p s w", w=W),
            1.0,
            dw_bc,
            Alu.mult,
            Alu.mult,
            accum_out=accs_t[:, 33:34],
        )

        # ---- PSUM accumulators for the PE route ----
        psq_a = psum.tile([1, 512], f32)
        psq_b = psum.tile([1, 512], f32)
        psl_a = psum.tile([1, 512], f32)
        psl_b = psum.tile([1, 512], f32)

        stride = max(1, n_tiles // max(n_dve, 1))
        dve_set = set((k * stride + stride // 2) % n_tiles for k in range(n_dve))
        n_pe_seen = 0
        for i in range(n_tiles):
            ot = out1_q[i // 4][:, i % 4, :]
            lb = lab_q[i // 4][:, i % 4, :]
            if i in dve_set:
                labD = junkp.tile([128, L], f16, tag="labD")
                nc.vector.tensor_tensor(labD[:], lb, d_b[:], Alu.mult)
                junkA = junkp.tile([128, L], f16, tag="junkA")
                nc.vector.scalar_tensor_tensor(
                    junkA[:], ot, 1.0, bv_b[:], Alu.mult, Alu.mult,
                    accum_out=accs_t[:, i : i + 1],
                )
                junkB = junkp.tile([128, L], f16, tag="junkB")
                nc.vector.scalar_tensor_tensor(
                    junkB[:], labD[:], 1.0, ot, Alu.mult, Alu.mult,
                    accum_out=accs_t[:, 16 + i : 17 + i],
                )
            else:
                st, sp = n_pe_seen == 0, n_pe_seen == n_pe - 1
                n_pe_seen += 1
                ql = junkp.tile([128, L], f16, tag="ql")
                nc.vector.tensor_tensor(ql[:], ot, lb, Alu.mult)
                nc.tensor.matmul(psq_a[:], ones16[:], ot[:, 0:512], start=st, stop=sp)
                nc.tensor.matmul(psq_b[:], ones16[:], ot[:, 512:L], start=st, stop=sp)
                nc.tensor.matmul(psl_a[:], ones16[:], ql[:, 0:512], start=st, stop=sp)
                nc.tensor.matmul(psl_b[:], ones16[:], ql[:, 512:L], start=st, stop=sp)

        # ---- epilogue: weighted dots straight out of PSUM ----
        res_t = const.tile([1, 8], f32)
        nc.vector.memset(res_t[:], 0.0)
        if n_pe > 0:

            def dot(ps_ap, row_ap, k, tag):
                junk = junkp.tile([1, 512], f32, tag="junkdot" + tag)
                nc.vector.scalar_tensor_tensor(
                    junk[:], ps_ap, 1.0, row_ap, Alu.mult, Alu.mult,
                    accum_out=res_t[0:1, k : k + 1],
                )

            dot(psq_a[:], bv32_t[0:1, 0:512], 0, "1a")
            dot(psq_b[:], bv32_t[0:1, 512:L], 1, "1b")
            dot(psl_a[:], dd32_t[0:1, 0:512], 2, "2a")
            dot(psl_b[:], dd32_t[0:1, 512:L], 3, "2b")
        nc.gpsimd.dma_start(res_d[:], res_t[:])
        nc.gpsimd.dma_start(accs_d[:], accs_t[:])
        nc.gpsimd.dma_start(flag_d[:], flag_t[:])

    nc.compile()
    return nc


def _get_nc():
    if getattr(_compiled, "nc", None) is None:
        _compiled.nc = _build()
    return _compiled.nc


def _in_maps(output, labels):
    bv, dd = _reward_rows()
    rows16 = np.stack([bv, dd]).astype(np.float16)
    bv32 = bv.astype(np.float32).reshape(1, L)
    dd32 = dd.astype(np.float32).reshape(1, L)
    out1 = np.ascontiguousarray(output[:, :, 1], dtype=np.float16)
    lab = labels.astype(np.uint8)  # values are 0/1
    rp = ROWS_PER_CORE
    maps = []
    for c in range(N_CORES):
        o = output[c * rp : (c + 1) * rp]
        # window rows packed so partition p holds rows p*16 .. p*16+15
        w0 = np.ascontiguousarray(o[:, L - W :, 0], dtype=np.float32).reshape(128, -1)
        w1 = np.ascontiguousarray(o[:, L - W :, 1], dtype=np.float32).reshape(128, -1)
        wf = np.concatenate([w0, w1], axis=1)
        labw = lab[c * rp : (c + 1) * rp, L - W :].reshape(128, -1)
        maps.append(
            {
                "out1": out1[c * rp : (c + 1) * rp],
                "lab": lab[c * rp : (c + 1) * rp],
                "wf": wf,
                "labw": np.ascontiguousarray(labw),
                "rows16": rows16,
                "bv32": bv32,
                "dd32": dd32,
            }
        )
    return maps


def _host_fallback(output, labels):
    temp = output[:, :, 1] > output[:, :, 0]
    allones = temp.all(axis=1)
    z = ~temp
    last_zero = (L - 1) - np.argmax(z[:, ::-1], axis=1)
    idx = np.where(allones, L, last_zero)
    mask = np.arange(L)[None, :] <= idx[:, None]
    j = np.arange(L, dtype=np.float64)
    r1 = np.where(labels == 1, -1.0 / np.log2(j + 2.0), (j + 1.0) / ALPHA)
    return np.float32(
        (output[:, :, 1].astype(np.float64) * mask * r1).sum() / B
    )


def _combine(results, output, labels):
    total = 0.0
    suspicious = 0
    for c, r in enumerate(results):
        p = np.asarray(r["partial"], dtype=np.float64)[0]
        total += p[0] + p[1] + p[2] + p[3]
        accs = np.asarray(r["accs"], dtype=np.float64)
        total += accs[:, 0:32].sum() - accs[:, 32:34].sum()
        # rows flagged "no zero in window": genuine all-ones rows are handled
        # (tail = 0) but a row whose last zero is before the window is not —
        # recheck on host. Never fires for +-symmetric random inputs.
        flags = np.asarray(r["flags"], dtype=np.float64)
        if flags.max() > 0:
            rp = ROWS_PER_CORE
            o = output[c * rp : (c + 1) * rp]
            allones_rows = (o[:, :, 1] > o[:, :, 0]).all(axis=1)
            flagged = flags.reshape(-1) > 0  # row = p*16 + s
            suspicious += int((flagged & ~allones_rows).sum())
    if suspicious > 0:
        return _host_fallback(output, labels)
    return np.float32(total / B)


def kernel(output: np.ndarray, labels: np.ndarray) -> np.ndarray:
    from concourse.bass_utils import run_bass_kernel_spmd

    assert output.shape == (B, L, 2), output.shape
    nc = _get_nc()
    res = run_bass_kernel_spmd(
        nc, _in_maps(output, labels), core_ids=list(range(N_CORES))
    )
    return _combine(res.results, output, labels)


# revision 10
# speedup vs baseline: 1.4634x; 1.4272x over previous
"""BiCutLoss Trainium2 kernel (8-core data parallel over batch).

Reference semantics (B=16384, L=1024):
    temp[b,j]  = argmax(output[b,j,:])          # 1 iff out1 > out0 (ties -> 0)
    idx[b]     = L if row all-ones else index of last zero
    mask[b,j]  = j <= idx[b]
    r1[b,j]    = -1/log2(j+2)  if labels==1 else (j+1)/alpha
    loss       = sum(output[...,1] * mask * r1) / B

Restructuring: masked_sum = full_sum - tail_sum, with the tail (j > idx)
confined to the last W=32 columns whenever each row has a zero decision
there (P(violation) = 2^-32 per row; a per-row flag catches it and the
host falls back to exact numpy, so the kernel is correct for all inputs).

v3 layout:
  * TRANSPOSED main stream: out1.T as [1024(j), 2048(b)] bf16 in 8
    chunks of [128, 2048]. With j on partitions the per-column reward
    weights Bv[j], D[j] become per-partition scalars, which every
    engine can apply natively:
      - term1 = sum Bv[j]*out1: PE matmuls with bv_col [128,1] as the
        stationary vector, all 32 accumulated into ONE [1,512] PSUM
        (order of adds is irrelevant for a global sum), then a single
        ScalarE Identity+accum drain.
      - term2 = sum D[j]*out1*lab: ql = out1*lab as a pure-bf16 DVE
        tensor_tensor (2x rate), then ScalarE activation with
        scale=d_col and accum_out (fused weighted row-sum) for most
        chunks, PE d_col-matmul for the rest.
  * labels stream as u8 HBM -> bf16 SBUF via GpSimd cast-DMA (1 byte of
    traffic per element, but pure-bf16 compute downstream).
  * window pack (last W=32 cols, row-major [128, 16*32]): host sends
    diff = out0w-out1w in f32 (sign of an f32 subtraction is exact, so
    argmax decisions match the f32 reference bit-for-bit), out1w/labw
    as bf16/u8. Tail mask via segment-iota + segmented reduce_max +
    broadcast compare; tail sums via two fused stt+accum.
  * bf16 value quantization contributes ~3e-3 rel err vs the 2e-2
    tolerance (random-sign cancellation over 16M terms).
"""

import threading
from contextlib import ExitStack

import numpy as np

B, L = 16384, 1024
N_CORES = 8
ROWS_PER_CORE = B // N_CORES  # 2048
ALPHA = 0.65
W = 32  # tail window width
N_SEG = ROWS_PER_CORE // 128  # 16 window segments per partition
N_CHUNKS = L // 128  # 8 transposed chunks
PE_T2_CHUNKS = (7,)  # chunks whose term2 reduce runs on PE instead of ScalarE

_compiled = threading.local()


def _reward_rows():
    j = np.arange(L, dtype=np.float64)
    bv = (j + 1.0) / ALPHA
    d = -1.0 / np.log2(j + 2.0) - bv
    return bv, d


def _build(num_devices=N_CORES):
    import concourse.tile as tile
    from concourse import bacc, mybir

    f32 = mybir.dt.float32
    bf16 = mybir.dt.bfloat16
    u8 = mybir.dt.uint8
    Alu = mybir.AluOpType
    Act = mybir.ActivationFunctionType
    Ax = mybir.AxisListType

    RB = ROWS_PER_CORE  # 2048 batch rows per core
    WSEG = N_SEG * W  # 512 packed window width

    nc = bacc.Bacc(
        "TRN2",
        target_bir_lowering=False,
        debug=False,
        enable_asserts=True,
        num_devices=num_devices,
    )

    out1T_d = nc.dram_tensor("out1T", [L, RB], bf16, kind="ExternalInput").ap()
    labT_d = nc.dram_tensor("labT", [L, RB], u8, kind="ExternalInput").ap()
    wdiff_d = nc.dram_tensor("wdiff", [128, WSEG], f32, kind="ExternalInput").ap()
    w1_d = nc.dram_tensor("w1", [128, WSEG], bf16, kind="ExternalInput").ap()
    labw_d = nc.dram_tensor("labw", [128, WSEG], u8, kind="ExternalInput").ap()
    bvcol_d = nc.dram_tensor("bvcol", [128, N_CHUNKS], bf16, kind="ExternalInput").ap()
    dcol16_d = nc.dram_tensor("dcol16", [128, N_CHUNKS], bf16, kind="ExternalInput").ap()
    dcol32_d = nc.dram_tensor("dcol32", [128, N_CHUNKS], f32, kind="ExternalInput").ap()
    bvw_d = nc.dram_tensor("bvw", [1, W], bf16, kind="ExternalInput").ap()
    dw_d = nc.dram_tensor("dw", [1, W], bf16, kind="ExternalInput").ap()

    res_d = nc.dram_tensor("partial", [1, 8], f32, kind="ExternalOutput").ap()
    accs_d = nc.dram_tensor("accs", [128, 12], f32, kind="ExternalOutput").ap()
    flag_d = nc.dram_tensor("flags", [128, N_SEG], bf16, kind="ExternalOutput").ap()

    with tile.TileContext(nc) as tc, ExitStack() as ctx:
        const = ctx.enter_context(tc.tile_pool(name="const", bufs=1))
        inp = ctx.enter_context(tc.tile_pool(name="inp", bufs=4))
        lpool = ctx.enter_context(tc.tile_pool(name="lpool", bufs=4))
        work = ctx.enter_context(tc.tile_pool(name="work", bufs=4))
        junkp = ctx.enter_context(tc.tile_pool(name="junkp", bufs=4))
        psum = ctx.enter_context(tc.tile_pool(name="psum", bufs=1, space="PSUM"))

        # ---- main input streams first so DMA starts flowing ----
        out1_c = []
        for f in range(N_CHUNKS // 2):
            t = inp.tile([128, 2, RB], bf16, tag="out1c")
            nc.sync.dma_start(
                t[:],
                out1T_d[256 * f : 256 * (f + 1), :].rearrange(
                    "(k p) b -> p k b", p=128
                ),
            )
            out1_c.append(t)
        lab_c = []
        for f in range(N_CHUNKS // 2):
            t = lpool.tile([128, 2, RB], bf16, tag="labc")
            nc.gpsimd.dma_start(  # gpsimd queue: u8 -> bf16 cast DMA
                t[:],
                labT_d[256 * f : 256 * (f + 1), :].rearrange(
                    "(k p) b -> p k b", p=128
                ),
            )
            lab_c.append(t)

        # ---- window inputs + constants ----
        wdiff_t = const.tile([128, WSEG], f32)
        nc.scalar.dma_start(wdiff_t[:], wdiff_d[:])
        w1_t = const.tile([128, WSEG], bf16)
        nc.scalar.dma_start(w1_t[:], w1_d[:])
        labw_t = const.tile([128, WSEG], bf16)
        nc.gpsimd.dma_start(labw_t[:], labw_d[:])  # cast u8 -> bf16
        bvcol_t = const.tile([128, N_CHUNKS], bf16)
        nc.scalar.dma_start(bvcol_t[:], bvcol_d[:])
        dcol16_t = const.tile([128, N_CHUNKS], bf16)
        nc.scalar.dma_start(dcol16_t[:], dcol16_d[:])
        dcol32_t = const.tile([128, N_CHUNKS], f32)
        nc.scalar.dma_start(dcol32_t[:], dcol32_d[:])
        bvw_t = const.tile([128, W], bf16)
        nc.scalar.dma_start(bvw_t[:], bvw_d[:].to_broadcast((128, W)))
        dw_t = const.tile([128, W], bf16)
        nc.scalar.dma_start(dw_t[:], dw_d[:].to_broadcast((128, W)))
        gio = const.tile([128, WSEG], bf16)
        nc.gpsimd.iota(
            gio[:],
            [[0, N_SEG], [1, W]],
            base=1,
            channel_multiplier=0,
            allow_small_or_imprecise_dtypes=True,
        )
        accs_t = const.tile([128, 12], f32)
        nc.vector.memset(accs_t[:], 0.0)
        res_t = const.tile([1, 8], f32)
        nc.vector.memset(res_t[:], 0.0)
        flag_t = const.tile([128, N_SEG], bf16)

        # ---- window block ----
        seg3 = lambda ap: ap.rearrange("p (s w) -> p s w", w=W)
        ge = work.tile([128, WSEG], bf16, tag="ge")
        nc.vector.tensor_scalar(ge[:], wdiff_t[:], 0.0, None, Alu.is_ge)
        gi = work.tile([128, WSEG], bf16, tag="gi")
        nc.vector.tensor_tensor(gi[:], ge[:], gio[:], Alu.mult)
        last0 = const.tile([128, N_SEG], bf16)
        nc.vector.tensor_reduce(last0[:], seg3(gi[:]), Ax.X, Alu.max)
        nc.vector.tensor_scalar(flag_t[:], last0[:], 0.5, None, Alu.is_le)
        l0p = const.tile([128, N_SEG], bf16)
        nc.vector.scalar_tensor_tensor(
            l0p[:], flag_t[:], float(W), last0[:], Alu.mult, Alu.add
        )
        tm = work.tile([128, WSEG], bf16, tag="tm")
        nc.vector.tensor_tensor(
            seg3(tm[:]), seg3(gio[:]),
            l0p[:].to_broadcast((128, N_SEG, W)), Alu.is_gt,
        )
        u = work.tile([128, WSEG], bf16, tag="u")
        nc.vector.tensor_tensor(u[:], tm[:], w1_t[:], Alu.mult)
        bvw_bc = (
            bvw_t[:].rearrange("p (s w) -> p s w", s=1).to_broadcast((128, N_SEG, W))
        )
        dw_bc = (
            dw_t[:].rearrange("p (s w) -> p s w", s=1).to_broadcast((128, N_SEG, W))
        )
        junk1 = junkp.tile([128, WSEG], bf16, tag="junkw")
        nc.vector.scalar_tensor_tensor(
            seg3(junk1[:]), seg3(u[:]), 1.0, bvw_bc, Alu.mult, Alu.mult,
            accum_out=accs_t[:, 8:9],
        )
        q2 = work.tile([128, WSEG], bf16, tag="q2")
        nc.vector.tensor_tensor(q2[:], u[:], labw_t[:], Alu.mult)
        junk2 = junkp.tile([128, WSEG], bf16, tag="junkw")
        nc.vector.scalar_tensor_tensor(
            seg3(junk2[:]), seg3(q2[:]), 1.0, dw_bc, Alu.mult, Alu.mult,
            accum_out=accs_t[:, 9:10],
        )

        # ---- main terms over 8 transposed chunks ----
        psA = psum.tile([1, 512], f32)  # term1: sum_j Bv_j * out1
        psB = psum.tile([1, 512], f32)  # term2 (PE chunks): sum_j D_j * ql
        n_pe_t2 = len(PE_T2_CHUNKS)
        Q = RB // 512  # 4 quarters per chunk
        a_seen = 0
        b_seen = 0
        for c in range(N_CHUNKS):
            ot = out1_c[c // 2][:, c % 2, :]
            lb = lab_c[c // 2][:, c % 2, :]
            ql = junkp.tile([128, RB], bf16, tag="ql")
            nc.vector.tensor_tensor(ql[:], ot, lb, Alu.mult)
            for k in range(Q):
                nc.tensor.matmul(
                    psA[:], bvcol_t[:, c : c + 1], ot[:, 512 * k : 512 * (k + 1)],
                    start=(a_seen == 0), stop=(a_seen == N_CHUNKS * Q - 1),
                )
                a_seen += 1
            if c in PE_T2_CHUNKS:
                for k in range(Q):
                    nc.tensor.matmul(
                        psB[:], dcol16_t[:, c : c + 1],
                        ql[:, 512 * k : 512 * (k + 1)],
                        start=(b_seen == 0), stop=(b_seen == n_pe_t2 * Q - 1),
                    )
                    b_seen += 1
            else:
                junk = junkp.tile([128, RB], bf16, tag="junkact")
                nc.scalar.activation(
                    junk[:], ql[:], Act.Identity,
                    scale=dcol32_t[:, c : c + 1],
                    accum_out=accs_t[:, c : c + 1],
                )

        # ---- epilogue: drain the two PSUM row-vectors ----
        ej_a = junkp.tile([1, 512], f32, tag="ejA")
        nc.scalar.activation(ej_a[:], psA[:], Act.Identity, accum_out=res_t[0:1, 0:1])
        if n_pe_t2:
            ej_b = junkp.tile([1, 512], f32, tag="ejB")
            nc.scalar.activation(
                ej_b[:], psB[:], Act.Identity, accum_out=res_t[0:1, 1:2]
            )
        nc.gpsimd.dma_start(res_d[:], res_t[:])
        nc.gpsimd.dma_start(accs_d[:], accs_t[:])
        nc.gpsimd.dma_start(flag_d[:], flag_t[:])

    nc.compile()
    return nc


def _get_nc():
    if getattr(_compiled, "nc", None) is None:
        _compiled.nc = _build()
    return _compiled.nc


def _in_maps(output, labels):
    import ml_dtypes

    bf16 = ml_dtypes.bfloat16
    bv, dd = _reward_rows()
    rp = ROWS_PER_CORE
    lab8 = labels.astype(np.uint8)
    jcol = np.arange(L).reshape(N_CHUNKS, 128).T  # [128, 8]: j = 128*c + p
    bvcol = bv[jcol].astype(bf16)
    dcol16 = dd[jcol].astype(bf16)
    dcol32 = dd[jcol].astype(np.float32)
    bvw = bv[L - W :].astype(bf16).reshape(1, W)
    dw = dd[L - W :].astype(bf16).reshape(1, W)
    maps = []
    for c in range(N_CORES):
        o = output[c * rp : (c + 1) * rp]
        out1T = np.ascontiguousarray(o[:, :, 1].T, dtype=bf16)
        labT = np.ascontiguousarray(lab8[c * rp : (c + 1) * rp].T)
        w0 = o[:, L - W :, 0].astype(np.float32)
        w1f = o[:, L - W :, 1].astype(np.float32)
        wdiff = (w0 - w1f).reshape(128, -1)
        maps.append(
            {
                "out1T": out1T,
                "labT": labT,
                "wdiff": np.ascontiguousarray(wdiff),
                "w1": np.ascontiguousarray(w1f.reshape(128, -1).astype(bf16)),
                "labw": np.ascontiguousarray(
                    lab8[c * rp : (c + 1) * rp, L - W :].reshape(128, -1)
                ),
                "bvcol": bvcol,
                "dcol16": dcol16,
                "dcol32": dcol32,
                "bvw": bvw,
                "dw": dw,
            }
        )
    return maps


def _host_fallback(output, labels):
    temp = output[:, :, 1] > output[:, :, 0]
    allones = temp.all(axis=1)
    z = ~temp
    last_zero = (L - 1) - np.argmax(z[:, ::-1], axis=1)
    idx = np.where(allones, L, last_zero)
    mask = np.arange(L)[None, :] <= idx[:, None]
    j = np.arange(L, dtype=np.float64)
    r1 = np.where(labels == 1, -1.0 / np.log2(j + 2.0), (j + 1.0) / ALPHA)
    return np.float32(
        (output[:, :, 1].astype(np.float64) * mask * r1).sum() / B
    )


def _combine(results, output, labels):
    total = 0.0
    suspicious = 0
    for c, r in enumerate(results):
        p = np.asarray(r["partial"], dtype=np.float64)[0]
        accs = np.asarray(r["accs"], dtype=np.float64)
        main = p[0] + p[1] + accs[:, 0:8].sum()
        tail = accs[:, 8] .sum() + accs[:, 9].sum()
        total += main - tail
        # rows flagged "no zero decision in window": genuine all-ones rows
        # are fine (tail = 0) but a row whose last zero is before the
        # window is not — recheck on host. P = 2^-32 per row for random
        # inputs.
        flags = np.asarray(r["flags"], dtype=np.float64)
        if flags.max() > 0:
            rp = ROWS_PER_CORE
            o = output[c * rp : (c + 1) * rp]
            allones_rows = (o[:, :, 1] > o[:, :, 0]).all(axis=1)
            flagged = flags.reshape(-1) > 0  # row = p*16 + s
            suspicious += int((flagged & ~allones_rows).sum())
    if suspicious > 0:
        return _host_fallback(output, labels)
    return np.float32(total / B)


def kernel(output: np.ndarray, labels: np.ndarray) -> np.ndarray:
    from concourse.bass_utils import run_bass_kernel_spmd

    assert output.shape == (B, L, 2), output.shape
    nc = _get_nc()
    res = run_bass_kernel_spmd(
        nc, _in_maps(output, labels), core_ids=list(range(N_CORES))
    )
    return _combine(res.results, output, labels)


# revision 12
# speedup vs baseline: 1.4689x; 1.0038x over previous
"""BiCutLoss Trainium2 kernel (8-core data parallel over batch).

Reference semantics (B=16384, L=1024):
    temp[b,j]  = argmax(output[b,j,:])          # 1 iff out1 > out0 (ties -> 0)
    idx[b]     = L if row all-ones else index of last zero
    mask[b,j]  = j <= idx[b]
    r1[b,j]    = -1/log2(j+2)  if labels==1 else (j+1)/alpha
    loss       = sum(output[...,1] * mask * r1) / B

Restructuring: masked_sum = full_sum - tail_sum, with the tail (j > idx)
confined to the last W=32 columns whenever each row has a zero decision
there (P(violation) = 2^-32 per row; a per-row flag catches it and the
host falls back to exact numpy, so the kernel stays correct for all
inputs).

v4 layout (engine-measured costs in ns for [128,2048] chunks):
  * TRANSPOSED main stream: out1.T as [1024(j), 2048(b)] bf16 in 8
    chunks of [128, 2048]. With j on partitions the per-column reward
    weights Bv[j], D[j] are per-partition scalars usable by every
    engine:
      - term1 = sum Bv[j]*out1: PE matmuls, bv_col [128,1] stationary,
        all 32 accumulated into ONE [1,512] PSUM, drained by a single
        ScalarE Identity+accum.
      - term2 = sum D[j]*out1*lab: ql = out1*lab on DVE (mixed
        bf16*u8, 2284ns), then ScalarE activation scale=d_col +
        accum_out (2080ns) for chunks 0-5 and PE d_col-matmuls for
        chunks 6-7 (they arrive last; the PE path drains fastest).
  * labels stream as raw u8 (no cast DMA: a u8->bf16 cast doubles DMA
    write traffic, and DMA at ~341 GB/s/core is the binding resource).
  * window pack (last W=32 cols, row-major [128, 16*32]): host sends
    diff = out0w-out1w in f32 (sign of f32 subtract is exact, so
    decisions match the reference), out1w bf16, labw u8 cast-DMA'd to
    bf16 (tiny). Tail mask via segment-iota + segmented reduce_max +
    broadcast compare; simple pure-bf16 multiplies offloaded to GpSimd.
  * bf16 value quantization contributes ~1e-4..3e-3 rel err vs the
    2e-2 tolerance (random-sign cancellation over 16M terms).
"""

import threading
from contextlib import ExitStack

import numpy as np

B, L = 16384, 1024
N_CORES = 8
ROWS_PER_CORE = B // N_CORES  # 2048
ALPHA = 0.65
W = 32  # tail window width
N_SEG = ROWS_PER_CORE // 128  # 16 window segments per partition
N_CHUNKS = L // 128  # 8 transposed chunks
PE_T2_CHUNKS = (6, 7)  # term2 on PE for these chunks (arrive last)
GPSIMD_QL_CHUNKS = (0,)  # ql on GpSimd for these chunks (rate experiment)

_compiled = threading.local()


def _reward_rows():
    j = np.arange(L, dtype=np.float64)
    bv = (j + 1.0) / ALPHA
    d = -1.0 / np.log2(j + 2.0) - bv
    return bv, d


def _build(num_devices=N_CORES):
    import concourse.tile as tile
    from concourse import bacc, mybir

    f32 = mybir.dt.float32
    bf16 = mybir.dt.bfloat16
    u8 = mybir.dt.uint8
    Alu = mybir.AluOpType
    Act = mybir.ActivationFunctionType
    Ax = mybir.AxisListType

    RB = ROWS_PER_CORE  # 2048 batch rows per core
    WSEG = N_SEG * W  # 512 packed window width

    nc = bacc.Bacc(
        "TRN2",
        target_bir_lowering=False,
        debug=False,
        enable_asserts=True,
        num_devices=num_devices,
    )

    out1T_d = nc.dram_tensor("out1T", [L, RB], bf16, kind="ExternalInput").ap()
    labT_d = nc.dram_tensor("labT", [L, RB], u8, kind="ExternalInput").ap()
    wdiff_d = nc.dram_tensor("wdiff", [128, WSEG], f32, kind="ExternalInput").ap()
    w1_d = nc.dram_tensor("w1", [128, WSEG], bf16, kind="ExternalInput").ap()
    labw_d = nc.dram_tensor("labw", [128, WSEG], u8, kind="ExternalInput").ap()
    bvcol_d = nc.dram_tensor("bvcol", [128, N_CHUNKS], bf16, kind="ExternalInput").ap()
    dcol16_d = nc.dram_tensor("dcol16", [128, N_CHUNKS], bf16, kind="ExternalInput").ap()
    dcol32_d = nc.dram_tensor("dcol32", [128, N_CHUNKS], f32, kind="ExternalInput").ap()
    bvw_d = nc.dram_tensor("bvw", [1, W], bf16, kind="ExternalInput").ap()
    dw_d = nc.dram_tensor("dw", [1, W], bf16, kind="ExternalInput").ap()

    res_d = nc.dram_tensor("partial", [1, 8], f32, kind="ExternalOutput").ap()
    accs_d = nc.dram_tensor("accs", [128, 28], f32, kind="ExternalOutput").ap()

    with tile.TileContext(nc) as tc, ExitStack() as ctx:
        const = ctx.enter_context(tc.tile_pool(name="const", bufs=1))
        inp = ctx.enter_context(tc.tile_pool(name="inp", bufs=4))
        lpool = ctx.enter_context(tc.tile_pool(name="lpool", bufs=4))
        work = ctx.enter_context(tc.tile_pool(name="work", bufs=4))
        junkp = ctx.enter_context(tc.tile_pool(name="junkp", bufs=4))
        psum = ctx.enter_context(tc.tile_pool(name="psum", bufs=1, space="PSUM"))

        # ---- main input streams first: big DMAs across queues ----
        out1_c = []
        for f in range(4):
            t = inp.tile([128, 2, RB], bf16, tag="out1c")
            nc.sync.dma_start(
                t[:],
                out1T_d[256 * f : 256 * (f + 1), :].rearrange(
                    "(k p) b -> p k b", p=128
                ),
            )
            out1_c.append(t)
        lab_c = []
        for f in range(4):
            t = lpool.tile([128, 2, RB], u8, tag="labc")
            nc.scalar.dma_start(
                t[:],
                labT_d[256 * f : 256 * (f + 1), :].rearrange(
                    "(k p) b -> p k b", p=128
                ),
            )
            lab_c.append(t)

        # ---- window inputs + constants (gpsimd queue) ----
        wdiff_t = const.tile([128, WSEG], f32)
        nc.gpsimd.dma_start(wdiff_t[:], wdiff_d[:])
        w1_t = const.tile([128, WSEG], bf16)
        nc.gpsimd.dma_start(w1_t[:], w1_d[:])
        labw_t = const.tile([128, WSEG], bf16)
        nc.gpsimd.dma_start(labw_t[:], labw_d[:])  # cast u8 -> bf16 (tiny)
        bvcol_t = const.tile([128, N_CHUNKS], bf16)
        nc.gpsimd.dma_start(bvcol_t[:], bvcol_d[:])
        dcol16_t = const.tile([128, N_CHUNKS], bf16)
        nc.gpsimd.dma_start(dcol16_t[:], dcol16_d[:])
        dcol32_t = const.tile([128, N_CHUNKS], f32)
        nc.gpsimd.dma_start(dcol32_t[:], dcol32_d[:])
        bvw_t = const.tile([128, W], bf16)
        nc.gpsimd.dma_start(bvw_t[:], bvw_d[:].to_broadcast((128, W)))
        dw_t = const.tile([128, W], bf16)
        nc.gpsimd.dma_start(dw_t[:], dw_d[:].to_broadcast((128, W)))
        gio = const.tile([128, WSEG], bf16)
        nc.gpsimd.iota(
            gio[:],
            [[0, N_SEG], [1, W]],
            base=1,
            channel_multiplier=0,
            allow_small_or_imprecise_dtypes=True,
        )
        accs_t = const.tile([128, 28], f32)
        nc.vector.memset(accs_t[:], 0.0)
        res_t = const.tile([1, 8], f32)
        nc.vector.memset(res_t[:], 0.0)

        # ---- window block ----
        seg3 = lambda ap: ap.rearrange("p (s w) -> p s w", w=W)
        ge = work.tile([128, WSEG], bf16, tag="ge")
        nc.vector.tensor_scalar(ge[:], wdiff_t[:], 0.0, None, Alu.is_ge)
        gi = work.tile([128, WSEG], bf16, tag="gi")
        nc.gpsimd.tensor_tensor(gi[:], ge[:], gio[:], Alu.mult)
        last0 = const.tile([128, N_SEG], bf16)
        nc.vector.tensor_reduce(last0[:], seg3(gi[:]), Ax.X, Alu.max)
        # flag (as f32, straight into the accs output block)
        nc.vector.tensor_scalar(
            accs_t[:, 12 : 12 + N_SEG], last0[:], 0.5, None, Alu.is_le
        )
        l0p = const.tile([128, N_SEG], bf16)
        nc.vector.scalar_tensor_tensor(
            l0p[:], accs_t[:, 12 : 12 + N_SEG], float(W), last0[:],
            Alu.mult, Alu.add,
        )
        tm = work.tile([128, WSEG], bf16, tag="tm")
        nc.vector.tensor_tensor(
            seg3(tm[:]), seg3(gio[:]),
            l0p[:].to_broadcast((128, N_SEG, W)), Alu.is_gt,
        )
        u = work.tile([128, WSEG], bf16, tag="u")
        nc.gpsimd.tensor_tensor(u[:], tm[:], w1_t[:], Alu.mult)
        q2 = work.tile([128, WSEG], bf16, tag="q2")
        nc.gpsimd.tensor_tensor(q2[:], u[:], labw_t[:], Alu.mult)
        bvw_bc = (
            bvw_t[:].rearrange("p (s w) -> p s w", s=1).to_broadcast((128, N_SEG, W))
        )
        dw_bc = (
            dw_t[:].rearrange("p (s w) -> p s w", s=1).to_broadcast((128, N_SEG, W))
        )
        junk1 = junkp.tile([128, WSEG], bf16, tag="junkw")
        nc.vector.scalar_tensor_tensor(
            seg3(junk1[:]), seg3(u[:]), 1.0, bvw_bc, Alu.mult, Alu.mult,
            accum_out=accs_t[:, 8:9],
        )
        junk2 = junkp.tile([128, WSEG], bf16, tag="junkw")
        nc.vector.scalar_tensor_tensor(
            seg3(junk2[:]), seg3(q2[:]), 1.0, dw_bc, Alu.mult, Alu.mult,
            accum_out=accs_t[:, 9:10],
        )

        # ---- main terms over 8 transposed chunks ----
        psA = psum.tile([1, 512], f32)  # term1: sum_j Bv_j * out1
        psB = psum.tile([1, 512], f32)  # term2 (PE chunks): sum_j D_j * ql
        n_pe_t2 = len(PE_T2_CHUNKS)
        Q = RB // 512  # 4 quarters per chunk
        a_seen = 0
        b_seen = 0
        for c in range(N_CHUNKS):
            ot = out1_c[c // 2][:, c % 2, :]
            lb = lab_c[c // 2][:, c % 2, :]
            ql = junkp.tile([128, RB], bf16, tag="ql")
            if c in GPSIMD_QL_CHUNKS:
                nc.gpsimd.tensor_tensor(ql[:], ot, lb, Alu.mult)
            else:
                nc.vector.tensor_tensor(ql[:], ot, lb, Alu.mult)
            for k in range(Q):
                nc.tensor.matmul(
                    psA[:], bvcol_t[:, c : c + 1], ot[:, 512 * k : 512 * (k + 1)],
                    start=(a_seen == 0), stop=(a_seen == N_CHUNKS * Q - 1),
                )
                a_seen += 1
            if c in PE_T2_CHUNKS:
                for k in range(Q):
                    nc.tensor.matmul(
                        psB[:], dcol16_t[:, c : c + 1],
                        ql[:, 512 * k : 512 * (k + 1)],
                        start=(b_seen == 0), stop=(b_seen == n_pe_t2 * Q - 1),
                    )
                    b_seen += 1
            else:
                junk = junkp.tile([128, RB], bf16, tag="junkact")
                nc.scalar.activation(
                    junk[:], ql[:], Act.Identity,
                    scale=dcol32_t[:, c : c + 1],
                    accum_out=accs_t[:, c : c + 1],
                )

        # ---- epilogue: drain the two PSUM row-vectors ----
        ej_a = junkp.tile([1, 512], f32, tag="ejA")
        nc.scalar.activation(ej_a[:], psA[:], Act.Identity, accum_out=res_t[0:1, 0:1])
        if n_pe_t2:
            ej_b = junkp.tile([1, 512], f32, tag="ejB")
            nc.scalar.activation(
                ej_b[:], psB[:], Act.Identity, accum_out=res_t[0:1, 1:2]
            )
        nc.sync.dma_start(res_d[:], res_t[:])
        nc.scalar.dma_start(accs_d[:], accs_t[:])

    nc.compile()
    return nc


def _get_nc():
    if getattr(_compiled, "nc", None) is None:
        _compiled.nc = _build()
    return _compiled.nc


def _in_maps(output, labels):
    import ml_dtypes

    bf16 = ml_dtypes.bfloat16
    bv, dd = _reward_rows()
    rp = ROWS_PER_CORE
    lab8 = labels.astype(np.uint8)
    jcol = np.arange(L).reshape(N_CHUNKS, 128).T  # [128, 8]: j = 128*c + p
    bvcol = bv[jcol].astype(bf16)
    dcol16 = dd[jcol].astype(bf16)
    dcol32 = dd[jcol].astype(np.float32)
    bvw = bv[L - W :].astype(bf16).reshape(1, W)
    dw = dd[L - W :].astype(bf16).reshape(1, W)
    maps = []
    for c in range(N_CORES):
        o = output[c * rp : (c + 1) * rp]
        out1T = np.ascontiguousarray(o[:, :, 1].T, dtype=bf16)
        labT = np.ascontiguousarray(lab8[c * rp : (c + 1) * rp].T)
        w0 = o[:, L - W :, 0].astype(np.float32)
        w1f = o[:, L - W :, 1].astype(np.float32)
        wdiff = (w0 - w1f).reshape(128, -1)
        maps.append(
            {
                "out1T": out1T,
                "labT": labT,
                "wdiff": np.ascontiguousarray(wdiff),
                "w1": np.ascontiguousarray(w1f.reshape(128, -1).astype(bf16)),
                "labw": np.ascontiguousarray(
                    lab8[c * rp : (c + 1) * rp, L - W :].reshape(128, -1)
                ),
                "bvcol": bvcol,
                "dcol16": dcol16,
                "dcol32": dcol32,
                "bvw": bvw,
                "dw": dw,
            }
        )
    return maps


def _host_fallback(output, labels):
    temp = output[:, :, 1] > output[:, :, 0]
    allones = temp.all(axis=1)
    z = ~temp
    last_zero = (L - 1) - np.argmax(z[:, ::-1], axis=1)
    idx = np.where(allones, L, last_zero)
    mask = np.arange(L)[None, :] <= idx[:, None]
    j = np.arange(L, dtype=np.float64)
    r1 = np.where(labels == 1, -1.0 / np.log2(j + 2.0), (j + 1.0) / ALPHA)
    return np.float32(
        (output[:, :, 1].astype(np.float64) * mask * r1).sum() / B
    )


def _combine(results, output, labels):
    total = 0.0
    suspicious = 0
    for c, r in enumerate(results):
        p = np.asarray(r["partial"], dtype=np.float64)[0]
        accs = np.asarray(r["accs"], dtype=np.float64)
        main = p[0] + p[1] + accs[:, 0:8].sum()
        tail = accs[:, 8].sum() + accs[:, 9].sum()
        total += main - tail
        # rows flagged "no zero decision in window": genuine all-ones rows
        # are fine (tail = 0) but a row whose last zero is before the
        # window is not — recheck on host. P = 2^-32 per row for random
        # inputs.
        flags = accs[:, 12 : 12 + N_SEG]
        if flags.max() > 0:
            rp = ROWS_PER_CORE
            o = output[c * rp : (c + 1) * rp]
            allones_rows = (o[:, :, 1] > o[:, :, 0]).all(axis=1)
            flagged = flags.reshape(-1) > 0  # row = p*16 + s
            suspicious += int((flagged & ~allones_rows).sum())
    if suspicious > 0:
        return _host_fallback(output, labels)
    return np.float32(total / B)


def kernel(output: np.ndarray, labels: np.ndarray) -> np.ndarray:
    from concourse.bass_utils import run_bass_kernel_spmd

    assert output.shape == (B, L, 2), output.shape
    nc = _get_nc()
    res = run_bass_kernel_spmd(
        nc, _in_maps(output, labels), core_ids=list(range(N_CORES))
    )
    return _combine(res.results, output, labels)
